# revision 2
# baseline (speedup 1.0000x reference)
"""Trainium2 kernel for nn_BLInputLayer — 4-queue SWDGE gather.

The baseline's dma_gather was bound by Q7 descriptor generation on a single
core-pair (queue 0) at ~7.9ns/row, with the Pool engine serializing every
chunk. Measured on HW: a dma_gather on queue_num q runs on core pair
(2q, 2q+1); instructions for queues 1-3 dispatch in ~120ns without blocking
the Pool engine and run concurrently, while queue-0 instructions block until
done. So: split the 32768 rows into 4 queue-quarters x 4 chunks of 2048,
dispatch queues 1-3 first (non-blocking), then queue 0's chunks (each blocks
the engine but pair 0 streams them back to back). All 4 pairs generate
descriptors concurrently => ~4x descriptor throughput, landing near the
HBM roofline. Stores are issued per-chunk from both HWDGE engines (SP for
queues 0-1, ACT for queues 2-3) as soon as each chunk's gather lands.

Dedup bookkeeping: host plans indices from coords (integer work only);
output row u gathers feats[src[u]] where src = first occurrence of the
u-th smallest key. Duplicate-point sums (~4 rows/batch) are added on the
host during result packing, mirroring the baseline's host-side crow math.
"""

import numpy as np

B, L, DIM, C = 8, 32768, 3, 128
S = 512
P = 128
NQ = 4                  # SWDGE queues / Q7 core pairs
QROWS = L // NQ         # 8192 rows per queue
# uniform small rounds: the SWDGE doorbell rings at chunk end, so smaller
# chunks start draining sooner and keep HBM busy during generation; 1024
# tokens also stays well inside the per-lane descriptor ring
TS = [1024] * 7 + [512, 512]
assert sum(TS) == QROWS and all(t % P == 0 for t in TS)
NCHQ = len(TS)
RSTART = [4 * sum(TS[:i]) for i in range(NCHQ)]      # row start of round i
TPPOFF = [sum(TS[:i]) // P for i in range(NCHQ)]     # gt tile offset of round i
SINGLE_PACKET = False
DMA_SCRATCH = 65536


def _build_nc():
    from concourse import bacc, mybir
    from concourse.library_config import mlp

    nc = bacc.Bacc("TRN2", target_bir_lowering=False, debug=False, num_devices=B,
                   dynamic_dma_scratch_size=DMA_SCRATCH, num_swdge_queues=NQ)
    f32, i16 = mybir.dt.float32, mybir.dt.int16
    feats = nc.dram_tensor("feats", [L, C], f32, kind="ExternalInput")
    gidx = nc.dram_tensor("gidx", [P, L // 16], i16, kind="ExternalInput")
    out = nc.dram_tensor("out", [L, C], f32, kind="ExternalOutput")

    # round-major layout: round i's chunks are contiguous in the output
    def rows(q, i):
        a = RSTART[i] + q * TS[i]
        return slice(a, a + TS[i])

    def cols(q, i):
        r = rows(q, i)
        return slice(r.start // 16, r.stop // 16)

    def gslice(gt, q, i):
        return gt[:, q, TPPOFF[i]:TPPOFF[i] + TS[i] // P]

    from contextlib import ExitStack

    with (
        nc.Block() as block,
        nc.sbuf_tensor("gidx_sb", [P, L // 16], i16) as gidx_sb,
        nc.sbuf_tensor("gt", [P, NQ, QROWS // P, C], f32) as gt,
        nc.sbuf_tensor("scrap", [P, 32], f32) as scrap,
        nc.semaphore("io") as io,
        nc.semaphore("ws") as ws,
        ExitStack() as stack,
    ):
        # one gather-completion semaphore per chunk: a shared per-queue
        # counter would race (a fast DMA engine finishing chunk i+1 can
        # satisfy 16*(i+1) while a slow engine is still on chunk i)
        gsem = [
            [
                stack.enter_context(nc.semaphore(f"g{_q}{_i}"))  # noqa: ANT232
                for _i in range(NCHQ)
            ]
            for _q in range(NQ)
        ]

        @block.gpsimd
        def _(gpsimd):
            gpsimd.load_library(mlp)
            # partition_broadcast is an mlp-library compute op with no input
            # dependency: dispatching it first forces the mlp ucode IRAM load
            # to overlap the gidx DMA instead of delaying the first gather
            gpsimd.partition_broadcast(scrap[:], scrap[0:1, :])
            gpsimd.wait_ge(io, 16)
            # per round: queues 1-3 dispatch without blocking the Pool engine;
            # the queue-0 instruction blocks until pair 0 finishes it, which
            # paces dispatch to one round per chunk-time with all 4 pairs busy
            for i in range(NCHQ):
                for q in (1, 2, 3, 0):
                    gpsimd.dma_gather(
                        gslice(gt, q, i), feats[:], gidx_sb[:, cols(q, i)],
                        TS[i], TS[i], C, single_packet=SINGLE_PACKET,
                        queue_num=q,
                    ).then_inc(gsem[q][i], 16)
            gpsimd.wait_ge(ws, 16 * NQ * NCHQ)

        def store_prog(eng, queues):
            for i in range(NCHQ):
                for q in queues:
                    eng.wait_ge(gsem[q][i], 16)
                    eng.dma_start(
                        out[rows(q, i), :].rearrange("(p t) c -> p (t c)", p=P),
                        gslice(gt, q, i),
                    ).then_inc(ws, 16)

        @block.sync
        def _(sync):
            sync.dma_start(gidx_sb[:], gidx[:]).then_inc(io, 16)
            store_prog(sync, (1, 0))

        @block.scalar
        def _(scalar):
            store_prog(scalar, (2, 3))

    nc.compile()
    return nc


_NC_CACHE = {}
_LAST_RESULTS = {}


def _plan_batch(coords_b):
    """Host-side integer planning from coords only. coords_b: [L,3] int32."""
    x = coords_b[:, 0].astype(np.int64)
    y = coords_b[:, 1].astype(np.int64)
    z = coords_b[:, 2].astype(np.int64)
    keys = ((x * S + y) * S + z).astype(np.int32)
    uniq, first_idx, inv = np.unique(keys, return_index=True, return_inverse=True)
    U = len(uniq)
    src = np.zeros(L, dtype=np.int64)
    src[:U] = first_idx
    # token j of a chunk fetches the row for slot (j%P)*tpp + j//P so each
    # partition holds tpp consecutive slots -> contiguous p-major writeback
    gidx = np.zeros((P, L // 16), np.int16)
    for i in range(NCHQ):
        tpp = TS[i] // P
        j = np.arange(TS[i])
        slot_local = (j % P) * tpp + j // P
        for q in range(NQ):
            a = RSTART[i] + q * TS[i]
            tokens = src[a + slot_local]
            w = tokens.reshape(TS[i] // 16, 16).T.astype(np.int16)
            gidx[:, a // 16:(a + TS[i]) // 16] = np.tile(w, (8, 1))
    dup_mask = np.ones(L, bool)
    dup_mask[first_idx] = False
    dup_points = np.nonzero(dup_mask)[0]
    dup_slots = inv[dup_points]
    return dict(U=U, gidx=gidx, dup_points=dup_points, dup_slots=dup_slots)


def kernel(coords, features):
    from concourse.bass_utils import run_bass_kernel_spmd

    coords = np.asarray(coords)
    features = np.ascontiguousarray(np.asarray(features, dtype=np.float32))
    plans = [_plan_batch(coords[b]) for b in range(B)]

    if 'nc' not in _NC_CACHE:
        _NC_CACHE['nc'] = _build_nc()
    nc = _NC_CACHE['nc']

    in_maps = [{"feats": features[b], "gidx": plans[b]['gidx']}
               for b in range(B)]

    import os
    trace = bool(os.environ.get("KERNEL_TRACE_DIR"))
    kw = {}
    if trace:
        try:
            import sys, types
            import antenv
            from trn_agent_boot.trn_boot import _ntff_profile_via_ctypes
            _h = _ntff_profile_via_ctypes('/opt/axon/libaxon_pjrt.so')
            mod = types.ModuleType('antenv.axon_hooks')
            mod.get_axon_ntff_profile_hook = (
                lambda: (lambda outdir, ids: _h(outdir, None)))
            mod.set_axon_ntff_profile_hook = lambda h: None
            sys.modules['antenv.axon_hooks'] = mod
            antenv.axon_hooks = mod
            import concourse.bass_utils as _bu
            _bu.upload_artifacts = lambda tmpdir: tmpdir
            os.makedirs(os.environ["KERNEL_TRACE_DIR"], exist_ok=True)
            kw = dict(trace=True, trace_cores=[0],
                      tmpdir=os.environ["KERNEL_TRACE_DIR"])
        except Exception:
            kw = {}

    res = None
    for attempt in range(3):
        try:
            res = run_bass_kernel_spmd(nc, in_maps, core_ids=list(range(B)), **kw)
            break
        except Exception:
            # transient NRT exec-unit errors recover on the next attempt
            if attempt == 2:
                raise
    _LAST_RESULTS['exec_time_ns'] = res.exec_time_ns

    full = np.zeros((B * L, C), np.float32)
    off = 0
    for b in range(B):
        U = plans[b]['U']
        full[off:off + U] = res.results[b]["out"][:U]
        dp, dslot = plans[b]['dup_points'], plans[b]['dup_slots']
        if len(dp):
            np.add.at(full, off + dslot, features[b][dp])
        off += U
    return full


# revision 3
# speedup vs baseline: 1.2385x; 1.2385x over previous
"""Trainium2 kernel for nn_BLInputLayer — 4-queue SWDGE gather.

The baseline's dma_gather was bound by Q7 descriptor generation on a single
core-pair (queue 0) at ~7.9ns/row, with the Pool engine serializing every
chunk. Measured on HW: a dma_gather on queue_num q runs on core pair
(2q, 2q+1); instructions for queues 1-3 dispatch in ~120ns without blocking
the Pool engine and run concurrently, while queue-0 instructions block until
done. So: split the 32768 rows across the 4 queues, in rounds of [1024]*7+[512]*2
tokens per queue; each round dispatches queues 1-3 (non-blocking) then queue
0 (blocks the Pool engine for the round's duration, which paces dispatch
while all 4 pairs work). All 4 pairs generate descriptors concurrently =>
~4x descriptor throughput; the SWDGE doorbell fires at chunk end, so the
small rounds keep the 16 SDMA engines draining throughout generation and
the kernel sits at the random-512B-read HBM ceiling (~300-400 GB/s
aggregate). Stores are issued per-chunk from both HWDGE rings (SP for
queues 1,0; ACT for queues 2,3) as soon as each chunk's gather lands,
gated by per-chunk semaphores (a shared counter would race).

Dedup bookkeeping: host plans indices from coords (integer work only);
output row u gathers feats[src[u]] where src = first occurrence of the
u-th smallest key. Duplicate-point sums (~4 rows/batch) are added on the
host during result packing, mirroring the baseline's host-side crow math.
"""

import numpy as np

B, L, DIM, C = 8, 32768, 3, 128
S = 512
P = 128
NQ = 4                  # SWDGE queues / Q7 core pairs
QROWS = L // NQ         # 8192 rows per queue
# uniform small rounds: the SWDGE doorbell rings at chunk end, so smaller
# chunks start draining sooner and keep HBM busy during generation; 1024
# tokens also stays well inside the per-lane descriptor ring
TS = [1024] * 7 + [512, 512]
assert sum(TS) == QROWS and all(t % P == 0 for t in TS)
NCHQ = len(TS)
RSTART = [4 * sum(TS[:i]) for i in range(NCHQ)]      # row start of round i
TPPOFF = [sum(TS[:i]) // P for i in range(NCHQ)]     # gt tile offset of round i
SINGLE_PACKET = False
DMA_SCRATCH = 65536


def _build_nc():
    from concourse import bacc, mybir
    from concourse.library_config import mlp

    nc = bacc.Bacc("TRN2", target_bir_lowering=False, debug=False, num_devices=B,
                   dynamic_dma_scratch_size=DMA_SCRATCH, num_swdge_queues=NQ)
    f32, i16 = mybir.dt.float32, mybir.dt.int16
    feats = nc.dram_tensor("feats", [L, C], f32, kind="ExternalInput")
    gidx = nc.dram_tensor("gidx", [P, L // 16], i16, kind="ExternalInput")
    out = nc.dram_tensor("out", [L, C], f32, kind="ExternalOutput")

    # round-major layout: round i's chunks are contiguous in the output
    def rows(q, i):
        a = RSTART[i] + q * TS[i]
        return slice(a, a + TS[i])

    def cols(q, i):
        r = rows(q, i)
        return slice(r.start // 16, r.stop // 16)

    def gslice(gt, q, i):
        return gt[:, q, TPPOFF[i]:TPPOFF[i] + TS[i] // P]

    from contextlib import ExitStack

    with (
        nc.Block() as block,
        nc.sbuf_tensor("gidx_sb", [P, L // 16], i16) as gidx_sb,
        nc.sbuf_tensor("gt", [P, NQ, QROWS // P, C], f32) as gt,
        nc.sbuf_tensor("scrap", [P, 32], f32) as scrap,
        nc.semaphore("io") as io,
        nc.semaphore("ws") as ws,
        ExitStack() as stack,
    ):
        # one gather-completion semaphore per chunk: a shared per-queue
        # counter would race (a fast DMA engine finishing chunk i+1 can
        # satisfy 16*(i+1) while a slow engine is still on chunk i)
        gsem = [
            [
                stack.enter_context(nc.semaphore(f"g{_q}{_i}"))  # noqa: ANT232
                for _i in range(NCHQ)
            ]
            for _q in range(NQ)
        ]

        @block.gpsimd
        def _(gpsimd):
            gpsimd.load_library(mlp)
            # partition_broadcast is an mlp-library compute op with no input
            # dependency: dispatching it first forces the mlp ucode IRAM load
            # to overlap the gidx DMA instead of delaying the first gather
            gpsimd.partition_broadcast(scrap[:], scrap[0:1, :])
            gpsimd.wait_ge(io, 16)
            # per round: queues 1-3 dispatch without blocking the Pool engine;
            # the queue-0 instruction blocks until pair 0 finishes it, which
            # paces dispatch to one round per chunk-time with all 4 pairs busy
            for i in range(NCHQ):
                for q in (1, 2, 3, 0):
                    gpsimd.dma_gather(
                        gslice(gt, q, i), feats[:], gidx_sb[:, cols(q, i)],
                        TS[i], TS[i], C, single_packet=SINGLE_PACKET,
                        queue_num=q,
                    ).then_inc(gsem[q][i], 16)
            gpsimd.wait_ge(ws, 16 * NQ * NCHQ)

        def store_prog(eng, queues):
            for i in range(NCHQ):
                for q in queues:
                    eng.wait_ge(gsem[q][i], 16)
                    eng.dma_start(
                        out[rows(q, i), :].rearrange("(p t) c -> p (t c)", p=P),
                        gslice(gt, q, i),
                    ).then_inc(ws, 16)

        @block.sync
        def _(sync):
            sync.dma_start(gidx_sb[:], gidx[:]).then_inc(io, 16)
            store_prog(sync, (1, 0))

        @block.scalar
        def _(scalar):
            store_prog(scalar, (2, 3))

    nc.compile()
    return nc


_NC_CACHE = {}
_LAST_RESULTS = {}


def _plan_batch(coords_b):
    """Host-side integer planning from coords only. coords_b: [L,3] int32."""
    x = coords_b[:, 0].astype(np.int64)
    y = coords_b[:, 1].astype(np.int64)
    z = coords_b[:, 2].astype(np.int64)
    keys = ((x * S + y) * S + z).astype(np.int32)
    uniq, first_idx, inv = np.unique(keys, return_index=True, return_inverse=True)
    U = len(uniq)
    src = np.zeros(L, dtype=np.int64)
    src[:U] = first_idx
    # token j of a chunk fetches the row for slot (j%P)*tpp + j//P so each
    # partition holds tpp consecutive slots -> contiguous p-major writeback
    gidx = np.zeros((P, L // 16), np.int16)
    for i in range(NCHQ):
        tpp = TS[i] // P
        j = np.arange(TS[i])
        slot_local = (j % P) * tpp + j // P
        for q in range(NQ):
            a = RSTART[i] + q * TS[i]
            tokens = src[a + slot_local]
            w = tokens.reshape(TS[i] // 16, 16).T.astype(np.int16)
            gidx[:, a // 16:(a + TS[i]) // 16] = np.tile(w, (8, 1))
    dup_mask = np.ones(L, bool)
    dup_mask[first_idx] = False
    dup_points = np.nonzero(dup_mask)[0]
    dup_slots = inv[dup_points]
    return dict(U=U, gidx=gidx, dup_points=dup_points, dup_slots=dup_slots)


def kernel(coords, features):
    from concourse.bass_utils import run_bass_kernel_spmd

    coords = np.asarray(coords)
    features = np.ascontiguousarray(np.asarray(features, dtype=np.float32))
    plans = [_plan_batch(coords[b]) for b in range(B)]

    if 'nc' not in _NC_CACHE:
        _NC_CACHE['nc'] = _build_nc()
    nc = _NC_CACHE['nc']

    in_maps = [{"feats": features[b], "gidx": plans[b]['gidx']}
               for b in range(B)]

    import os
    trace = bool(os.environ.get("KERNEL_TRACE_DIR"))
    kw = {}
    if trace:
        try:
            import sys, types
            import antenv
            from trn_agent_boot.trn_boot import _ntff_profile_via_ctypes
            _h = _ntff_profile_via_ctypes('/opt/axon/libaxon_pjrt.so')
            mod = types.ModuleType('antenv.axon_hooks')
            mod.get_axon_ntff_profile_hook = (
                lambda: (lambda outdir, ids: _h(outdir, None)))
            mod.set_axon_ntff_profile_hook = lambda h: None
            sys.modules['antenv.axon_hooks'] = mod
            antenv.axon_hooks = mod
            import concourse.bass_utils as _bu
            _bu.upload_artifacts = lambda tmpdir: tmpdir
            os.makedirs(os.environ["KERNEL_TRACE_DIR"], exist_ok=True)
            kw = dict(trace=True, trace_cores=[0],
                      tmpdir=os.environ["KERNEL_TRACE_DIR"])
        except Exception:
            kw = {}

    res = None
    for attempt in range(3):
        try:
            res = run_bass_kernel_spmd(nc, in_maps, core_ids=list(range(B)), **kw)
            break
        except Exception:
            # transient NRT exec-unit errors recover on the next attempt
            if attempt == 2:
                raise
    _LAST_RESULTS['exec_time_ns'] = res.exec_time_ns

    full = np.zeros((B * L, C), np.float32)
    off = 0
    for b in range(B):
        U = plans[b]['U']
        full[off:off + U] = res.results[b]["out"][:U]
        dp, dslot = plans[b]['dup_points'], plans[b]['dup_slots']
        if len(dp):
            np.add.at(full, off + dslot, features[b][dp])
        off += U
    return full


# revision 4
# speedup vs baseline: 1.2556x; 1.0138x over previous
"""Trainium2 kernel for nn_BLInputLayer — 4-queue SWDGE gather.

The baseline's dma_gather was bound by Q7 descriptor generation on a single
core-pair (queue 0) at ~7.9ns/row, with the Pool engine serializing every
chunk. Measured on HW: a dma_gather on queue_num q runs on core pair
(2q, 2q+1); instructions for queues 1-3 dispatch in ~120ns without blocking
the Pool engine and run concurrently, while queue-0 instructions block until
done. So: split the 32768 rows across the 4 queues, in rounds of [1024]*7+[512]*2
tokens per queue; each round dispatches queues 1-3 (non-blocking) then queue
0 (blocks the Pool engine for the round's duration, which paces dispatch
while all 4 pairs work). All 4 pairs generate descriptors concurrently =>
~4x descriptor throughput; the SWDGE doorbell fires at chunk end, so the
small rounds keep the 16 SDMA engines draining throughout generation and
the kernel sits at the random-512B-read HBM ceiling (~300-400 GB/s
aggregate). Stores are issued per-chunk from both HWDGE rings (SP for
queues 1,0; ACT for queues 2,3) as soon as each chunk's gather lands,
gated by per-chunk semaphores (a shared counter would race).

Dedup bookkeeping: host plans indices from coords (integer work only);
output row u gathers feats[src[u]] where src = first occurrence of the
u-th smallest key. Duplicate-point sums (~4 rows/batch) are added on the
host during result packing, mirroring the baseline's host-side crow math.
"""

import numpy as np

B, L, DIM, C = 8, 32768, 3, 128
S = 512
P = 128
NQ = 4                  # SWDGE queues / Q7 core pairs
QROWS = L // NQ         # 8192 rows per queue
# small rounds: the SWDGE doorbell rings at chunk end, so smaller chunks
# start draining sooner; per-queue first/last sizes are staggered so the
# four queues' doorbells interleave instead of firing in lockstep, which
# smooths aggregate descriptor supply to the SDMA engines
TSQ = [
    [512] + [1024] * 7 + [512],
    [640] + [1024] * 7 + [384],
    [768] + [1024] * 7 + [256],
    [896] + [1024] * 7 + [128],
]
assert all(sum(t) == QROWS and all(x % P == 0 for x in t) for t in TSQ)
NCHQ = len(TSQ[0])
# queue q owns output rows [QROWS*q, QROWS*(q+1)), chunked by TSQ[q]
QOFF = [[sum(TSQ[q][:i]) for i in range(NCHQ)] for q in range(NQ)]
SINGLE_PACKET = False
DMA_SCRATCH = 65536


def _build_nc():
    from concourse import bacc, mybir
    from concourse.library_config import mlp

    nc = bacc.Bacc("TRN2", target_bir_lowering=False, debug=False, num_devices=B,
                   dynamic_dma_scratch_size=DMA_SCRATCH, num_swdge_queues=NQ)
    f32, i16 = mybir.dt.float32, mybir.dt.int16
    feats = nc.dram_tensor("feats", [L, C], f32, kind="ExternalInput")
    gidx = nc.dram_tensor("gidx", [P, L // 16], i16, kind="ExternalInput")
    out = nc.dram_tensor("out", [L, C], f32, kind="ExternalOutput")

    # queue-major layout: queue q's chunks are contiguous in the output
    def rows(q, i):
        a = QROWS * q + QOFF[q][i]
        return slice(a, a + TSQ[q][i])

    def cols(q, i):
        r = rows(q, i)
        return slice(r.start // 16, r.stop // 16)

    def gslice(gt, q, i):
        return gt[:, q, QOFF[q][i] // P:(QOFF[q][i] + TSQ[q][i]) // P]

    from contextlib import ExitStack

    with (
        nc.Block(no_gpsimd_drain=True) as block,
        nc.sbuf_tensor("gidx_sb", [P, L // 16], i16) as gidx_sb,
        nc.sbuf_tensor("gt", [P, NQ, QROWS // P, C], f32) as gt,
        nc.sbuf_tensor("scrap", [P, 32], f32) as scrap,
        nc.semaphore("io") as io,
        nc.semaphore("ws") as ws,
        ExitStack() as stack,
    ):
        # one gather-completion semaphore per chunk: a shared per-queue
        # counter would race (a fast DMA engine finishing chunk i+1 can
        # satisfy 16*(i+1) while a slow engine is still on chunk i)
        gsem = [
            [
                stack.enter_context(nc.semaphore(f"g{_q}{_i}"))  # noqa: ANT232
                for _i in range(NCHQ)
            ]
            for _q in range(NQ)
        ]

        @block.gpsimd
        def _(gpsimd):
            gpsimd.load_library(mlp)
            # partition_broadcast is an mlp-library compute op with no input
            # dependency: dispatching it first forces the mlp ucode IRAM load
            # to overlap the gidx DMA instead of delaying the first gather
            gpsimd.partition_broadcast(scrap[:], scrap[0:1, :])
            gpsimd.wait_ge(io, 16)
            # per round: queues 1-3 dispatch without blocking the Pool engine;
            # the queue-0 instruction blocks until pair 0 finishes it, which
            # paces dispatch to one round per chunk-time with all 4 pairs busy
            for i in range(NCHQ):
                for q in (1, 2, 3, 0):
                    gpsimd.dma_gather(
                        gslice(gt, q, i), feats[:], gidx_sb[:, cols(q, i)],
                        TSQ[q][i], TSQ[q][i], C, single_packet=SINGLE_PACKET,
                        queue_num=q,
                    ).then_inc(gsem[q][i], 16)
            gpsimd.wait_ge(ws, 16 * NQ * NCHQ)

        def store_prog(eng, queues):
            for i in range(NCHQ):
                for q in queues:
                    eng.wait_ge(gsem[q][i], 16)
                    eng.dma_start(
                        out[rows(q, i), :].rearrange("(p t) c -> p (t c)", p=P),
                        gslice(gt, q, i),
                    ).then_inc(ws, 16)

        @block.sync
        def _(sync):
            sync.dma_start(gidx_sb[:], gidx[:]).then_inc(io, 16)
            store_prog(sync, (1, 0))

        @block.scalar
        def _(scalar):
            store_prog(scalar, (2, 3))

    nc.compile()
    return nc


_NC_CACHE = {}
_LAST_RESULTS = {}


def _plan_batch(coords_b):
    """Host-side integer planning from coords only. coords_b: [L,3] int32."""
    x = coords_b[:, 0].astype(np.int64)
    y = coords_b[:, 1].astype(np.int64)
    z = coords_b[:, 2].astype(np.int64)
    keys = ((x * S + y) * S + z).astype(np.int32)
    uniq, first_idx, inv = np.unique(keys, return_index=True, return_inverse=True)
    U = len(uniq)
    src = np.zeros(L, dtype=np.int64)
    src[:U] = first_idx
    # token j of a chunk fetches the row for slot (j%P)*tpp + j//P so each
    # partition holds tpp consecutive slots -> contiguous p-major writeback
    gidx = np.zeros((P, L // 16), np.int16)
    for q in range(NQ):
        for i in range(NCHQ):
            t = TSQ[q][i]
            tpp = t // P
            j = np.arange(t)
            slot_local = (j % P) * tpp + j // P
            a = QROWS * q + QOFF[q][i]
            tokens = src[a + slot_local]
            w = tokens.reshape(t // 16, 16).T.astype(np.int16)
            gidx[:, a // 16:(a + t) // 16] = np.tile(w, (8, 1))
    dup_mask = np.ones(L, bool)
    dup_mask[first_idx] = False
    dup_points = np.nonzero(dup_mask)[0]
    dup_slots = inv[dup_points]
    return dict(U=U, gidx=gidx, dup_points=dup_points, dup_slots=dup_slots)


def kernel(coords, features):
    from concourse.bass_utils import run_bass_kernel_spmd

    coords = np.asarray(coords)
    features = np.ascontiguousarray(np.asarray(features, dtype=np.float32))
    plans = [_plan_batch(coords[b]) for b in range(B)]

    if 'nc' not in _NC_CACHE:
        _NC_CACHE['nc'] = _build_nc()
    nc = _NC_CACHE['nc']

    in_maps = [{"feats": features[b], "gidx": plans[b]['gidx']}
               for b in range(B)]

    import os
    trace = bool(os.environ.get("KERNEL_TRACE_DIR"))
    kw = {}
    if trace:
        try:
            import sys, types
            import antenv
            from trn_agent_boot.trn_boot import _ntff_profile_via_ctypes
            _h = _ntff_profile_via_ctypes('/opt/axon/libaxon_pjrt.so')
            mod = types.ModuleType('antenv.axon_hooks')
            mod.get_axon_ntff_profile_hook = (
                lambda: (lambda outdir, ids: _h(outdir, None)))
            mod.set_axon_ntff_profile_hook = lambda h: None
            sys.modules['antenv.axon_hooks'] = mod
            antenv.axon_hooks = mod
            import concourse.bass_utils as _bu
            _bu.upload_artifacts = lambda tmpdir: tmpdir
            os.makedirs(os.environ["KERNEL_TRACE_DIR"], exist_ok=True)
            kw = dict(trace=True, trace_cores=[0],
                      tmpdir=os.environ["KERNEL_TRACE_DIR"])
        except Exception:
            kw = {}

    res = None
    for attempt in range(3):
        try:
            res = run_bass_kernel_spmd(nc, in_maps, core_ids=list(range(B)), **kw)
            break
        except Exception:
            # transient NRT exec-unit errors recover on the next attempt
            if attempt == 2:
                raise
    _LAST_RESULTS['exec_time_ns'] = res.exec_time_ns

    full = np.zeros((B * L, C), np.float32)
    off = 0
    for b in range(B):
        U = plans[b]['U']
        full[off:off + U] = res.results[b]["out"][:U]
        dp, dslot = plans[b]['dup_points'], plans[b]['dup_slots']
        if len(dp):
            np.add.at(full, off + dslot, features[b][dp])
        off += U
    return full
